# revision 10
# baseline (speedup 1.0000x reference)
"""Trainium2 Bass kernel for nn_BoundarySeg (gnn_message_passing).

Computation (per example b, position j, MAX_SPAN_LEN=6 window):
    first[j]  = sum_{d=0..5, j+d<L} w[j, j+d] * h[j+d]
    second[j] = h[j] * sum_{d, j+d<L} w[j, j+d]
    out[j]    = concat([first, second])            # [B, L, 2H]

Only the 6-diagonal band of the [B, L, L] adjacency is ever used, so the
host extracts that band (a pure strided gather / data-layout step) and
builds small banded weight matrices; all arithmetic (the windowed weighted
sums and the scaled copy) runs on-device.

Device strategy (pure data parallel, B=16 sharded 2-per-core over 8 cores):
  - Valid-window tiling: output tile t covers rows [123t, 123t+123) and
    consumes h rows [123t, 123t+128), so every 6-wide window lies inside
    one K=128 contraction block — a single banded matmul per tile
    (lhsT[k, m] = band[123t+m, k-m]), no cross-tile seams.
  - `second` as a per-partition tensor_scalar multiply on the Vector
    engine, with the window sums reduced on-device from the band.
  - h tiles are plain row slices of the unmodified bound_hidden tensor
    (9 overlapping 128-row DMAs per example); banded weights + band are
    one small packed DMA per example.
  - HBM traffic per core ~20 MB (in ~7.5 MB + out 12.6 MB): memory-bound.
"""

import os
import sys

import numpy as np

if "/opt/trn_rl_repo" not in sys.path:
    sys.path.insert(0, "/opt/trn_rl_repo")

B, L, H = 16, 1024, 768
D = 6             # MAX_SPAN_LEN
NCORES = 8
BP = B // NCORES  # examples per core
P = 128
MT = 123          # output rows per tile (valid window: MT + D - 1 <= P)
NT = 9            # ceil(L / MT)

WA_COLS = NT * MT           # 1107
BAND_OFF = WA_COLS          # band block starts here
WB_F = WA_COLS + NT * D     # 1161 total cols in wband tensor

_nc_cache = None


def _tile_dims(t):
    m = min(MT, L - MT * t)          # output rows in tile t (123 or 40)
    k = min(P, L - MT * t)           # h rows consumed (128 or 40)
    return m, k


def _build_bass():
    import concourse.tile as tile
    from concourse import bacc, mybir

    f32 = mybir.dt.float32
    nc = bacc.Bacc("TRN2", target_bir_lowering=False)

    h_d = nc.dram_tensor("h", [BP, L, H], f32, kind="ExternalInput")
    wband_d = nc.dram_tensor("wband", [BP, P, WB_F], f32, kind="ExternalInput")
    out_d = nc.dram_tensor("out", [BP, L, 2 * H], f32, kind="ExternalOutput")

    with tile.TileContext(nc) as tc:
        with (
            tc.tile_pool(name="hpool", bufs=4) as hpool,
            tc.tile_pool(name="wpool", bufs=2) as wpool,
            tc.tile_pool(name="opool", bufs=4) as opool,
            tc.tile_pool(name="spool", bufs=4) as spool,
            tc.tile_pool(name="pspool", bufs=4, space="PSUM") as pspool,
        ):
            for ex in range(BP):
                wband = wpool.tile([P, WB_F], f32)
                nc.sync.dma_start(out=wband, in_=wband_d[ex])

                for t in range(NT):
                    m, k = _tile_dims(t)
                    h_sb = hpool.tile([P, H], f32)
                    nc.sync.dma_start(
                        out=h_sb[0:k, :], in_=h_d[ex, MT * t : MT * t + k, :]
                    )
                    lhsT = wband[0:k, t * MT : t * MT + m]
                    psum = pspool.tile([P, H], f32)
                    # fp32 matmul: moving operand <= 512 cols (one PSUM bank)
                    for c0, c1 in ((0, 512), (512, H)):
                        nc.tensor.matmul(
                            out=psum[0:m, c0:c1],
                            lhsT=lhsT,
                            rhs=h_sb[0:k, c0:c1],
                            start=True,
                            stop=True,
                        )
                    out_sb = opool.tile([P, 2 * H], f32)
                    nc.scalar.copy(out=out_sb[0:m, 0:H], in_=psum[0:m, :])
                    wsum = spool.tile([P, 1], f32)
                    nc.vector.reduce_sum(
                        out=wsum[0:m],
                        in_=wband[0:m, BAND_OFF + t * D : BAND_OFF + (t + 1) * D],
                        axis=mybir.AxisListType.X,
                    )
                    nc.vector.tensor_scalar_mul(
                        out=out_sb[0:m, H : 2 * H],
                        in0=h_sb[0:m, :],
                        scalar1=wsum[0:m],
                    )
                    nc.sync.dma_start(
                        out=out_d[ex, MT * t : MT * t + m, :], in_=out_sb[0:m, :]
                    )
    nc.compile()
    return nc


def _host_prep(span_adjacency, bound_hidden):
    """Extract the used 6-wide diagonal band and pack the banded matmul
    weights. Pure gather/layout — no arithmetic on the data."""
    adj = span_adjacency.reshape(B, L, L)
    band = np.zeros((B, NT * MT, D), dtype=np.float32)
    for d in range(D):
        # band[b, j, d] = adj[b, j, j+d] for j+d < L, else 0
        band[:, : L - d, d] = np.diagonal(adj, offset=d, axis1=1, axis2=2)
    band_t = band.reshape(B, NT, MT, D)

    # lhsT[b, t, k, m] = band[b, 123t+m, k-m] for 0 <= k-m <= 5
    wa = np.zeros((B, NT, P, MT), dtype=np.float32)
    for d in range(D):
        mm = np.arange(MT)
        wa[:, :, mm + d, mm] = band_t[:, :, :, d]

    wband = np.empty((B, P, WB_F), dtype=np.float32)
    # wa block: [k, t*MT + m]
    wband[:, :, :WA_COLS] = wa.transpose(0, 2, 1, 3).reshape(B, P, NT * MT)
    # band block: [p, t*D + d] = band[123t+p, d] (p < 123 meaningful)
    wband[:, :, BAND_OFF:] = np.pad(
        band_t.transpose(0, 2, 1, 3), ((0, 0), (0, P - MT), (0, 0), (0, 0))
    ).reshape(B, P, NT * D)

    h = np.ascontiguousarray(bound_hidden, dtype=np.float32)
    return [
        {
            "h": np.ascontiguousarray(h[BP * c : BP * (c + 1)]),
            "wband": np.ascontiguousarray(wband[BP * c : BP * (c + 1)]),
        }
        for c in range(NCORES)
    ]


def run(span_adjacency, bound_hidden, trace=False):
    """Run on 8 NeuronCores; returns (out [B, L, 2H] f32, exec_time_ns|None)."""
    global _nc_cache
    from concourse import bass_utils

    in_maps = _host_prep(np.asarray(span_adjacency), np.asarray(bound_hidden))
    if _nc_cache is None:
        _nc_cache = _build_bass()
    res = bass_utils.run_bass_kernel_spmd(
        _nc_cache, in_maps, core_ids=list(range(NCORES)), trace=trace
    )
    out = np.concatenate([r["out"] for r in res.results], axis=0)
    return out, res.exec_time_ns


def kernel(span_adjacency, bound_hidden):
    out, _ = run(span_adjacency, bound_hidden, trace=False)
    return out


# revision 11
# speedup vs baseline: 2.2582x; 2.2582x over previous
"""Trainium2 Bass kernel for nn_BoundarySeg (gnn_message_passing).

Computation (per example b, position j, MAX_SPAN_LEN=6 window):
    first[j]  = sum_{d=0..5, j+d<L} w[j, j+d] * h[j+d]
    second[j] = h[j] * sum_{d, j+d<L} w[j, j+d]
    out[j]    = concat([first, second])            # [B, L, 2H]

Only the 6-diagonal band of the [B, L, L] adjacency is ever used, so the
host extracts that band (a pure strided gather / data-layout step) and
builds small banded weight matrices; all arithmetic (the windowed weighted
sums and the scaled copy) runs on-device.

Device strategy (pure data parallel, B=16 sharded 2-per-core over 8 cores):
  - Valid-window tiling: output tile t covers rows [123t, 123t+123) and
    consumes h rows [123t, 123t+128), so every 6-wide window lies inside
    one K=128 contraction block — a single banded matmul per tile
    (lhsT[k, m] = band[123t+m, k-m]), no cross-tile seams.
  - `second` as a per-partition tensor_scalar multiply on the Vector
    engine, with the window sums reduced on-device from the band.
  - h tiles are plain row slices of the unmodified bound_hidden tensor
    (9 overlapping 128-row DMAs per example); banded weights + band are
    one small packed DMA per example.
  - HBM traffic per core ~20 MB (in ~7.5 MB + out 12.6 MB): memory-bound.
"""

import os
import sys

import numpy as np

if "/opt/trn_rl_repo" not in sys.path:
    sys.path.insert(0, "/opt/trn_rl_repo")

B, L, H = 16, 1024, 768
D = 6             # MAX_SPAN_LEN
NCORES = 8
BP = B // NCORES  # examples per core
P = 128
MT = 112          # output rows per tile; multiple of 16 so the output DMA
                  # splits evenly across all 16 SDMA engines (a 123-row DMA
                  # gets balanced onto only 3 engines and runs ~5x slower)
KT = MT + D - 1   # 117 h rows consumed per tile (valid-window contraction)
NT = 10           # ceil(L / MT); last tile has 16 rows

WA_COLS = NT * MT           # 1120
BAND_OFF = WA_COLS          # band block starts here
WB_F = WA_COLS + NT * D     # 1180 total cols in wband tensor

_nc_cache = None


def _tile_dims(t):
    m = min(MT, L - MT * t)          # output rows in tile t (112 or 16)
    k = min(KT, L - MT * t)          # h rows contracted (117 or 16)
    kh = min(P, L - MT * t)          # h rows loaded (128 or 16; mult of 16)
    return m, k, kh


def _build_bass():
    import concourse.tile as tile
    from concourse import bacc, mybir

    f32 = mybir.dt.float32
    nc = bacc.Bacc("TRN2", target_bir_lowering=False)

    h_d = nc.dram_tensor("h", [BP, L, H], f32, kind="ExternalInput")
    wband_d = nc.dram_tensor("wband", [BP, P, WB_F], f32, kind="ExternalInput")
    out_d = nc.dram_tensor("out", [BP, L, 2 * H], f32, kind="ExternalOutput")

    with tile.TileContext(nc) as tc:
        with (
            tc.tile_pool(name="hpool", bufs=4) as hpool,
            tc.tile_pool(name="wpool", bufs=2) as wpool,
            tc.tile_pool(name="opool", bufs=4) as opool,
            tc.tile_pool(name="spool", bufs=4) as spool,
            tc.tile_pool(name="pspool", bufs=4, space="PSUM") as pspool,
        ):
            for ex in range(BP):
                wband = wpool.tile([P, WB_F], f32)
                nc.sync.dma_start(out=wband, in_=wband_d[ex])

                for t in range(NT):
                    m, k, kh = _tile_dims(t)
                    h_sb = hpool.tile([P, H], f32)
                    nc.sync.dma_start(
                        out=h_sb[0:kh, :], in_=h_d[ex, MT * t : MT * t + kh, :]
                    )
                    lhsT = wband[0:k, t * MT : t * MT + m]
                    psum = pspool.tile([P, H], f32)
                    # fp32 matmul: moving operand <= 512 cols (one PSUM bank)
                    for c0, c1 in ((0, 512), (512, H)):
                        nc.tensor.matmul(
                            out=psum[0:m, c0:c1],
                            lhsT=lhsT,
                            rhs=h_sb[0:k, c0:c1],
                            start=True,
                            stop=True,
                        )
                    out_sb = opool.tile([P, 2 * H], f32)
                    nc.scalar.copy(out=out_sb[0:m, 0:H], in_=psum[0:m, :])
                    wsum = spool.tile([P, 1], f32)
                    nc.vector.reduce_sum(
                        out=wsum[0:m],
                        in_=wband[0:m, BAND_OFF + t * D : BAND_OFF + (t + 1) * D],
                        axis=mybir.AxisListType.X,
                    )
                    nc.vector.tensor_scalar_mul(
                        out=out_sb[0:m, H : 2 * H],
                        in0=h_sb[0:m, :],
                        scalar1=wsum[0:m],
                    )
                    nc.sync.dma_start(
                        out=out_d[ex, MT * t : MT * t + m, :], in_=out_sb[0:m, :]
                    )
    nc.compile()
    return nc


def _host_prep(span_adjacency, bound_hidden):
    """Extract the used 6-wide diagonal band and pack the banded matmul
    weights. Pure gather/layout — no arithmetic on the data."""
    adj = span_adjacency.reshape(B, L, L)
    band = np.zeros((B, NT * MT, D), dtype=np.float32)
    for d in range(D):
        # band[b, j, d] = adj[b, j, j+d] for j+d < L, else 0
        band[:, : L - d, d] = np.diagonal(adj, offset=d, axis1=1, axis2=2)
    band_t = band.reshape(B, NT, MT, D)

    # lhsT[b, t, k, m] = band[b, 123t+m, k-m] for 0 <= k-m <= 5
    wa = np.zeros((B, NT, P, MT), dtype=np.float32)
    for d in range(D):
        mm = np.arange(MT)
        wa[:, :, mm + d, mm] = band_t[:, :, :, d]

    wband = np.empty((B, P, WB_F), dtype=np.float32)
    # wa block: [k, t*MT + m]
    wband[:, :, :WA_COLS] = wa.transpose(0, 2, 1, 3).reshape(B, P, NT * MT)
    # band block: [p, t*D + d] = band[123t+p, d] (p < 123 meaningful)
    wband[:, :, BAND_OFF:] = np.pad(
        band_t.transpose(0, 2, 1, 3), ((0, 0), (0, P - MT), (0, 0), (0, 0))
    ).reshape(B, P, NT * D)

    h = np.ascontiguousarray(bound_hidden, dtype=np.float32)
    return [
        {
            "h": np.ascontiguousarray(h[BP * c : BP * (c + 1)]),
            "wband": np.ascontiguousarray(wband[BP * c : BP * (c + 1)]),
        }
        for c in range(NCORES)
    ]


def run(span_adjacency, bound_hidden, trace=False):
    """Run on 8 NeuronCores; returns (out [B, L, 2H] f32, exec_time_ns|None)."""
    global _nc_cache
    from concourse import bass_utils

    in_maps = _host_prep(np.asarray(span_adjacency), np.asarray(bound_hidden))
    if _nc_cache is None:
        _nc_cache = _build_bass()
    res = bass_utils.run_bass_kernel_spmd(
        _nc_cache, in_maps, core_ids=list(range(NCORES)), trace=trace
    )
    out = np.concatenate([r["out"] for r in res.results], axis=0)
    return out, res.exec_time_ns


def kernel(span_adjacency, bound_hidden):
    out, _ = run(span_adjacency, bound_hidden, trace=False)
    return out


# revision 12
# speedup vs baseline: 2.3975x; 1.0617x over previous
"""Trainium2 Bass kernel for nn_BoundarySeg (gnn_message_passing).

Computation (per example b, position j, MAX_SPAN_LEN=6 window):
    first[j]  = sum_{d=0..5, j+d<L} w[j, j+d] * h[j+d]
    second[j] = h[j] * sum_{d, j+d<L} w[j, j+d]
    out[j]    = concat([first, second])            # [B, L, 2H]

Only the 6-diagonal band of the [B, L, L] adjacency is ever used, so the
host extracts that band (a pure strided gather / data-layout step) and
builds small banded weight matrices; all arithmetic (the windowed weighted
sums and the scaled copy) runs on-device.

Device strategy (pure data parallel, B=16 sharded 2-per-core over 8 cores):
  - Valid-window tiling, MT=112 output rows per tile: tile t consumes h
    rows [112t, 112t+128), so every 6-wide window lies inside one K=128
    contraction block — a single banded matmul per tile
    (lhsT[k, m] = band[112t+m, k-m], zero off the 6 diagonals), no seams.
  - `second` as a per-partition tensor_scalar multiply on the Vector
    engine, with the window sums reduced on-device from the band.
  - DMA efficiency: h is host-packed so partition p of block t holds row
    112t+p — chunked loads then carry >=9KB contiguous per-partition
    descriptor runs; the output uses a packed [112, NT*1536] DRAM layout
    (host un-packs) so two tiles share one 12KB-descriptor store. All DMA
    partition counts are multiples of 16 so the descriptor balancer
    spreads every transfer across all 16 SDMA engines.
  - HBM traffic per core ~21.6 MB (in ~9 MB + out 12.6 MB): memory-bound.
"""

import os
import sys

import numpy as np

if "/opt/trn_rl_repo" not in sys.path:
    sys.path.insert(0, "/opt/trn_rl_repo")

B, L, H = 16, 1024, 768
D = 6             # MAX_SPAN_LEN
NCORES = 8
BP = B // NCORES  # examples per core
P = 128
MT = 112          # output rows per tile; multiple of 16 so every DMA
                  # splits evenly across all 16 SDMA engines (e.g. a
                  # 123-row DMA lands on only 3 engines and is ~5x slower)
NT = 10           # ceil(L / MT); last tile has 16 output rows
HPAD = MT * (NT - 1) + P  # padded h rows for the packed layout (1136)

WA_COLS = NT * MT           # 1120
BAND_OFF = WA_COLS
WB_F = WA_COLS + NT * D     # 1180 cols in wband tensor

H_CHUNKS = ((0, 3), (3, 6), (6, 10))  # h-block DMA batching
OUT_PAIR = 2                           # output tiles per store DMA

_nc_cache = None


def _build_bass():
    import concourse.tile as tile
    from concourse import bacc, mybir

    f32 = mybir.dt.float32
    nc = bacc.Bacc("TRN2", target_bir_lowering=False)

    h_d = nc.dram_tensor("hpack", [BP, P, NT * H], f32, kind="ExternalInput")
    wband_d = nc.dram_tensor("wband", [BP, P, WB_F], f32, kind="ExternalInput")
    out_d = nc.dram_tensor("outpack", [BP, MT, NT * 2 * H], f32, kind="ExternalOutput")

    with tile.TileContext(nc) as tc:
        with (
            tc.tile_pool(name="hpool", bufs=3) as hpool,
            tc.tile_pool(name="wpool", bufs=2) as wpool,
            tc.tile_pool(name="opool", bufs=3) as opool,
            tc.tile_pool(name="spool", bufs=4) as spool,
            tc.tile_pool(name="pspool", bufs=4, space="PSUM") as pspool,
        ):
            for ex in range(BP):
                wband = wpool.tile([P, WB_F], f32)
                nc.sync.dma_start(out=wband, in_=wband_d[ex])

                h_tiles = {}
                for c0, c1 in H_CHUNKS:
                    hc = hpool.tile([P, (H_CHUNKS[-1][1] - H_CHUNKS[-1][0]) * H], f32)
                    nc.sync.dma_start(
                        out=hc[:, 0 : (c1 - c0) * H], in_=h_d[ex, :, c0 * H : c1 * H]
                    )
                    for t in range(c0, c1):
                        h_tiles[t] = hc[:, (t - c0) * H : (t - c0 + 1) * H]

                for pc in range(NT // OUT_PAIR):
                    out_sb = opool.tile([MT, OUT_PAIR * 2 * H], f32)
                    for tt in range(OUT_PAIR):
                        t = pc * OUT_PAIR + tt
                        m = min(MT, L - MT * t)  # 112, or 16 on the last tile
                        rhs = h_tiles[t]
                        lhsT = wband[:, t * MT : t * MT + m]
                        psum = pspool.tile([P, H], f32)
                        # fp32 matmul: moving operand <= 512 cols (one bank)
                        for c0, c1 in ((0, 512), (512, H)):
                            nc.tensor.matmul(
                                out=psum[0:m, c0:c1],
                                lhsT=lhsT,
                                rhs=rhs[:, c0:c1],
                                start=True,
                                stop=True,
                            )
                        ob = tt * 2 * H
                        nc.scalar.copy(
                            out=out_sb[0:m, ob : ob + H], in_=psum[0:m, :]
                        )
                        wsum = spool.tile([P, 1], f32)
                        nc.vector.reduce_sum(
                            out=wsum[0:m],
                            in_=wband[0:m, BAND_OFF + t * D : BAND_OFF + (t + 1) * D],
                            axis=mybir.AxisListType.X,
                        )
                        nc.vector.tensor_scalar_mul(
                            out=out_sb[0:m, ob + H : ob + 2 * H],
                            in0=rhs[0:m, :],
                            scalar1=wsum[0:m],
                        )
                    nc.sync.dma_start(
                        out=out_d[
                            ex, :, pc * OUT_PAIR * 2 * H : (pc + 1) * OUT_PAIR * 2 * H
                        ],
                        in_=out_sb,
                    )
    nc.compile()
    return nc


def _host_prep(span_adjacency, bound_hidden):
    """Extract the used 6-wide diagonal band, build the banded matmul
    weights, and pack h into the 112-stride block layout. Pure
    gather/layout — no arithmetic on the data."""
    adj = span_adjacency.reshape(B, L, L)
    band = np.zeros((B, NT * MT, D), dtype=np.float32)
    for d in range(D):
        # band[b, j, d] = adj[b, j, j+d] for j+d < L, else 0
        band[:, : L - d, d] = np.diagonal(adj, offset=d, axis1=1, axis2=2)
    band_t = band.reshape(B, NT, MT, D)

    # lhsT[b, t, k, m] = band[b, 112t+m, k-m] for 0 <= k-m <= 5
    wa = np.zeros((B, NT, P, MT), dtype=np.float32)
    mm = np.arange(MT)
    for d in range(D):
        wa[:, :, mm + d, mm] = band_t[:, :, :, d]

    wband = np.empty((B, P, WB_F), dtype=np.float32)
    wband[:, :, :WA_COLS] = wa.transpose(0, 2, 1, 3).reshape(B, P, NT * MT)
    wband[:, :, BAND_OFF:] = np.pad(
        band_t.transpose(0, 2, 1, 3), ((0, 0), (0, P - MT), (0, 0), (0, 0))
    ).reshape(B, P, NT * D)

    # packed h: partition p, block t holds row 112t+p (rows >= L are zero)
    h_pad = np.zeros((B, HPAD, H), dtype=np.float32)
    h_pad[:, :L] = bound_hidden
    idx = (MT * np.arange(NT)[:, None] + np.arange(P)[None, :]).ravel()
    hpack = (
        h_pad[:, idx, :].reshape(B, NT, P, H).transpose(0, 2, 1, 3).reshape(B, P, NT * H)
    )

    return [
        {
            "hpack": np.ascontiguousarray(hpack[BP * c : BP * (c + 1)]),
            "wband": np.ascontiguousarray(wband[BP * c : BP * (c + 1)]),
        }
        for c in range(NCORES)
    ]


def _host_unpack(outpacks):
    """outpack [BP, 112, NT*1536] per core -> out [B, L, 1536]."""
    op = np.concatenate(outpacks, axis=0)  # [B, 112, NT*1536]
    out = (
        op.reshape(B, MT, NT, 2 * H).transpose(0, 2, 1, 3).reshape(B, NT * MT, 2 * H)
    )
    return np.ascontiguousarray(out[:, :L])


def run(span_adjacency, bound_hidden, trace=False):
    """Run on 8 NeuronCores; returns (out [B, L, 2H] f32, exec_time_ns|None)."""
    global _nc_cache
    from concourse import bass_utils

    in_maps = _host_prep(np.asarray(span_adjacency), np.asarray(bound_hidden))
    if _nc_cache is None:
        _nc_cache = _build_bass()
    res = bass_utils.run_bass_kernel_spmd(
        _nc_cache, in_maps, core_ids=list(range(NCORES)), trace=trace
    )
    out = _host_unpack([r["outpack"] for r in res.results])
    return out, res.exec_time_ns


def kernel(span_adjacency, bound_hidden):
    out, _ = run(span_adjacency, bound_hidden, trace=False)
    return out


# revision 15
# speedup vs baseline: 2.4901x; 1.0386x over previous
"""Trainium2 Bass kernel for nn_BoundarySeg (gnn_message_passing).

Computation (per example b, position j, MAX_SPAN_LEN=6 window):
    first[j]  = sum_{d=0..5, j+d<L} w[j, j+d] * h[j+d]
    second[j] = h[j] * sum_{d, j+d<L} w[j, j+d]
    out[j]    = concat([first, second])            # [B, L, 2H]

Only the 6-diagonal band of the [B, L, L] adjacency is ever used, so the
host extracts that band (a pure strided gather / data-layout step) and
builds small banded weight matrices; all arithmetic (the windowed weighted
sums and the scaled copy) runs on-device.

Device strategy (pure data parallel, B=16 sharded 2-per-core over 8 cores):
  - 128-aligned h blocks; tile t computes out rows [128t, 128t+123) as one
    banded matmul (lhsT[k, m] = band[128t+m, k-m], zero off the diagonals)
    against h block t. The remaining 5 boundary rows per block (whose
    windows straddle the block edge) are computed by ONE batched
    block-diagonal matmul per example over the 10 consecutive h rows at
    each boundary (K=80, M=40), then placed into the staged output tiles
    with small SBUF->SBUF DMAs (engine partition-slices must be 32-aligned
    on TRN2; DMA has no such restriction).
  - `second` as a per-partition tensor_scalar multiply on the Vector
    engine, with the window sums reduced on-device from the band.
  - DMA efficiency: h and out use partition-major packed DRAM layouts
    (host packs/unpacks) giving 24KB / 12KB contiguous descriptor runs,
    and every DMA's partition count is a multiple of 16 so the descriptor
    balancer spreads each transfer across all 16 SDMA engines.
  - HBM traffic per core ~20 MB (in ~7.5 MB + out 12.6 MB): memory-bound.
"""

import os
import sys

import numpy as np

if "/opt/trn_rl_repo" not in sys.path:
    sys.path.insert(0, "/opt/trn_rl_repo")

B, L, H = 16, 1024, 768
D = 6             # MAX_SPAN_LEN
NCORES = 8
BP = B // NCORES  # examples per core
P = 128
NT = L // P       # 8 aligned tiles per example
MT = P - (D - 1)  # 123 main-matmul output rows per tile
SW = D - 1        # 5 boundary rows per block
SR = 2 * SW       # 10 h rows feeding each boundary group
SK = NT * SR      # 80 rows in the batched seam matmul (K)
SM = NT * SW      # 40 seam output rows (M)

# wband column layout
WA_COLS = NT * MT           # 984
BAND_OFF = WA_COLS          # 8*6 = 48 band cols
SEAM_OFF = BAND_OFF + NT * D
WB_F = SEAM_OFF + SM        # 1072

OUT_PAIR = 2                # output tiles per store DMA (12KB descriptors)

_nc_cache = None


def _build_bass():
    import concourse.tile as tile
    from concourse import bacc, mybir

    f32 = mybir.dt.float32
    nc = bacc.Bacc("TRN2", target_bir_lowering=False)

    h_d = nc.dram_tensor("hpack", [BP, P, NT * H], f32, kind="ExternalInput")
    hs_d = nc.dram_tensor("hseam", [BP, SK, H], f32, kind="ExternalInput")
    wband_d = nc.dram_tensor("wband", [BP, P, WB_F], f32, kind="ExternalInput")
    out_d = nc.dram_tensor("outpack", [BP, P, NT * 2 * H], f32, kind="ExternalOutput")

    with tile.TileContext(nc) as tc:
        with (
            tc.tile_pool(name="hpool", bufs=2) as hpool,
            tc.tile_pool(name="wpool", bufs=2) as wpool,
            tc.tile_pool(name="srhs", bufs=2) as srhs_pool,
            tc.tile_pool(name="ssb", bufs=2) as ssb_pool,
            tc.tile_pool(name="opool", bufs=4) as opool,
            tc.tile_pool(name="spool", bufs=4) as spool,
            tc.tile_pool(name="pspool", bufs=3, space="PSUM") as pspool,
            tc.tile_pool(name="pseam", bufs=1, space="PSUM") as pseam_pool,
        ):
            for ex in range(BP):
                wband = wpool.tile([P, WB_F], f32)
                nc.sync.dma_start(out=wband, in_=wband_d[ex])
                h_sb = hpool.tile([P, NT * H], f32)
                nc.sync.dma_start(out=h_sb, in_=h_d[ex])

                # boundary rows: one block-diagonal matmul over the 10
                # consecutive h rows at each block edge
                seam_rhs = srhs_pool.tile([SK, H], f32)
                nc.sync.dma_start(out=seam_rhs, in_=hs_d[ex])
                psum_seam = pseam_pool.tile([SM, H], f32)
                for c0, c1 in ((0, 512), (512, H)):
                    nc.tensor.matmul(
                        out=psum_seam[:, c0:c1],
                        lhsT=wband[0:SK, SEAM_OFF : SEAM_OFF + SM],
                        rhs=seam_rhs[:, c0:c1],
                        start=True,
                        stop=True,
                    )
                seam_sb = ssb_pool.tile([SM, H], f32)
                nc.scalar.copy(out=seam_sb, in_=psum_seam[:])

                for pc in range(NT // OUT_PAIR):
                    out_sb = opool.tile([P, OUT_PAIR * 2 * H], f32)
                    for tt in range(OUT_PAIR):
                        t = pc * OUT_PAIR + tt
                        rhs = h_sb[:, t * H : (t + 1) * H]
                        lhsT = wband[:, t * MT : (t + 1) * MT]
                        psum = pspool.tile([P, H], f32)
                        # fp32 matmul: moving operand <= 512 cols (one bank)
                        for c0, c1 in ((0, 512), (512, H)):
                            nc.tensor.matmul(
                                out=psum[0:MT, c0:c1],
                                lhsT=lhsT,
                                rhs=rhs[:, c0:c1],
                                start=True,
                                stop=True,
                            )
                        ob = tt * 2 * H
                        nc.scalar.copy(
                            out=out_sb[0:MT, ob : ob + H], in_=psum[0:MT, :]
                        )
                        # boundary rows 123..127 of this block
                        nc.sync.dma_start(
                            out=out_sb[MT:P, ob : ob + H],
                            in_=seam_sb[SW * t : SW * (t + 1), :],
                        )
                        wsum = spool.tile([P, 1], f32)
                        nc.vector.reduce_sum(
                            out=wsum,
                            in_=wband[:, BAND_OFF + t * D : BAND_OFF + (t + 1) * D],
                            axis=mybir.AxisListType.X,
                        )
                        nc.vector.tensor_scalar_mul(
                            out=out_sb[:, ob + H : ob + 2 * H],
                            in0=rhs,
                            scalar1=wsum,
                        )
                    nc.sync.dma_start(
                        out=out_d[
                            ex, :, pc * OUT_PAIR * 2 * H : (pc + 1) * OUT_PAIR * 2 * H
                        ],
                        in_=out_sb,
                    )
    nc.compile()
    return nc


def _host_prep(span_adjacency, bound_hidden):
    """Extract the used 6-wide diagonal band, build the banded matmul
    weights, and pack h partition-major. Pure gather/layout — no
    arithmetic on the data."""
    adj = span_adjacency.reshape(B, L, L)
    band = np.zeros((B, L, D), dtype=np.float32)
    for d in range(D):
        # band[b, j, d] = adj[b, j, j+d] for j+d < L, else 0
        band[:, : L - d, d] = np.diagonal(adj, offset=d, axis1=1, axis2=2)
    band_t = band.reshape(B, NT, P, D)

    # main lhsT[b, t, k, m] = band[b, 128t+m, k-m] for m < 123 (full windows)
    wa = np.zeros((B, NT, P, MT), dtype=np.float32)
    mm = np.arange(MT)
    for d in range(D):
        wa[:, :, mm + d, mm] = band_t[:, :, :MT, d]

    # seam lhsT[b, 10s+u, 5s+q] = band[b, 128s+123+q, u-q] for 0 <= u-q <= 5
    # (k row 10s+u is h row 128s+123+u; out row m=5s+q is j=128s+123+q)
    seam = np.zeros((B, SK, SM), dtype=np.float32)
    s = np.arange(NT)
    for q in range(SW):
        for u in range(q, q + D):
            seam[:, SR * s + u, SW * s + q] = band_t[:, s, MT + q, u - q]

    wband = np.zeros((B, P, WB_F), dtype=np.float32)
    wband[:, :, :WA_COLS] = wa.transpose(0, 2, 1, 3).reshape(B, P, NT * MT)
    wband[:, :, BAND_OFF:SEAM_OFF] = band_t.transpose(0, 2, 1, 3).reshape(B, P, NT * D)
    wband[:, :SK, SEAM_OFF:] = seam

    h32 = np.ascontiguousarray(bound_hidden, dtype=np.float32)
    # packed h: partition p, block t holds row 128t+p
    hpack = h32.reshape(B, NT, P, H).transpose(0, 2, 1, 3).reshape(B, P, NT * H)
    # seam h rows: 10 consecutive rows 128s+123 .. 128s+132 per boundary
    # (rows >= L are only multiplied by zero weights; use zeros)
    h_pad = np.zeros((B, NT * P + SR, H), dtype=np.float32)
    h_pad[:, :L] = h32
    idx = (P * np.arange(NT)[:, None] + MT + np.arange(SR)[None, :]).ravel()
    hseam = h_pad[:, idx, :]

    return [
        {
            "hpack": np.ascontiguousarray(hpack[BP * c : BP * (c + 1)]),
            "hseam": np.ascontiguousarray(hseam[BP * c : BP * (c + 1)]),
            "wband": np.ascontiguousarray(wband[BP * c : BP * (c + 1)]),
        }
        for c in range(NCORES)
    ]


def _host_unpack(outpacks):
    """outpack [BP, 128, NT*1536] per core -> out [B, L, 1536]."""
    op = np.concatenate(outpacks, axis=0)
    return np.ascontiguousarray(
        op.reshape(B, P, NT, 2 * H).transpose(0, 2, 1, 3).reshape(B, L, 2 * H)
    )


def run(span_adjacency, bound_hidden, trace=False):
    """Run on 8 NeuronCores; returns (out [B, L, 2H] f32, exec_time_ns|None)."""
    global _nc_cache
    from concourse import bass_utils

    in_maps = _host_prep(np.asarray(span_adjacency), np.asarray(bound_hidden))
    if _nc_cache is None:
        _nc_cache = _build_bass()
    res = bass_utils.run_bass_kernel_spmd(
        _nc_cache, in_maps, core_ids=list(range(NCORES)), trace=trace
    )
    out = _host_unpack([r["outpack"] for r in res.results])
    return out, res.exec_time_ns


def kernel(span_adjacency, bound_hidden):
    out, _ = run(span_adjacency, bound_hidden, trace=False)
    return out


# revision 17
# speedup vs baseline: 2.8248x; 1.1344x over previous
"""Trainium2 Bass kernel for nn_BoundarySeg (gnn_message_passing).

Computation (per example b, position j, MAX_SPAN_LEN=6 window):
    first[j]  = sum_{d=0..5, j+d<L} w[j, j+d] * h[j+d]
    second[j] = h[j] * sum_{d, j+d<L} w[j, j+d]
    out[j]    = concat([first, second])            # [B, L, 2H]

Only the 6-diagonal band of the [B, L, L] adjacency is ever used, so the
host extracts that band (a pure strided gather / data-layout step) and
builds small banded weight matrices; all arithmetic (the windowed weighted
sums and the scaled copy) runs on-device.

Device strategy (pure data parallel, B=16 sharded 2-per-core over 8 cores):
  - 128-aligned h blocks; tile t computes out rows [128t, 128t+123) as one
    banded matmul (lhsT[k, m] = band[128t+m, k-m], zero off the diagonals)
    against h block t. The remaining 5 boundary rows per block (whose
    windows straddle the block edge) are computed by ONE batched
    block-diagonal matmul per example over the 10 consecutive h rows at
    each boundary (K=80, M=40), then placed into the staged output tiles
    with small SBUF->SBUF DMAs (engine partition-slices must be 32-aligned
    on TRN2; DMA has no such restriction).
  - `second` as a per-partition tensor_scalar multiply on the Vector
    engine, with the window sums reduced on-device from the band.
  - DMA efficiency: h and out use partition-major packed DRAM layouts
    (host packs/unpacks) giving 24KB / 12KB contiguous descriptor runs,
    and every DMA's partition count is a multiple of 16 so the descriptor
    balancer spreads each transfer across all 16 SDMA engines.
  - HBM traffic per core ~20 MB (in ~7.5 MB + out 12.6 MB): memory-bound.
"""

import os
import sys

import numpy as np

if "/opt/trn_rl_repo" not in sys.path:
    sys.path.insert(0, "/opt/trn_rl_repo")

B, L, H = 16, 1024, 768
D = 6             # MAX_SPAN_LEN
NCORES = 8
BP = B // NCORES  # examples per core
P = 128
NT = L // P       # 8 aligned tiles per example
MT = P - (D - 1)  # 123 main-matmul output rows per tile
SW = D - 1        # 5 boundary rows per block
SR = 2 * SW       # 10 h rows feeding each boundary group
SK = NT * SR      # 80 rows in the batched seam matmul (K)
SM = NT * SW      # 40 seam output rows (M)

# wband column layout
WA_COLS = NT * MT           # 984
BAND_OFF = WA_COLS          # 8*6 = 48 band cols
SEAM_OFF = BAND_OFF + NT * D
WB_F = SEAM_OFF + SM        # 1072

OUT_PAIR = 2                # output tiles per store DMA (12KB descriptors)

_nc_cache = None


def _build_bass():
    import concourse.tile as tile
    from concourse import bacc, mybir

    f32 = mybir.dt.float32
    nc = bacc.Bacc("TRN2", target_bir_lowering=False)

    h_d = nc.dram_tensor("hpack", [BP, P, NT * H], f32, kind="ExternalInput")
    hs_d = nc.dram_tensor("hseam", [BP, SK, H], f32, kind="ExternalInput")
    wband_d = nc.dram_tensor("wband", [BP, P, WB_F], f32, kind="ExternalInput")
    out_d = nc.dram_tensor("outpack", [BP, P, NT * 2 * H], f32, kind="ExternalOutput")

    with tile.TileContext(nc) as tc:
        with (
            tc.tile_pool(name="hpool", bufs=2) as hpool,
            tc.tile_pool(name="wpool", bufs=2) as wpool,
            tc.tile_pool(name="srhs", bufs=2) as srhs_pool,
            tc.tile_pool(name="ssb", bufs=2) as ssb_pool,
            tc.tile_pool(name="opool", bufs=2) as opool,
            tc.tile_pool(name="spool", bufs=4) as spool,
            tc.tile_pool(name="pspool", bufs=3, space="PSUM") as pspool,
            tc.tile_pool(name="pseam", bufs=1, space="PSUM") as pseam_pool,
        ):
            for ex in range(BP):
                # h first: it is the long-pole input for the main matmuls.
                # DMA issue is serialized per DGE sequencer (~0.7us each),
                # so loads go on SP, stores on ACT, seam placement on SWDGE.
                h_sb = hpool.tile([P, NT * H], f32)
                for c0, c1 in ((0, NT // 2), (NT // 2, NT)):
                    nc.sync.dma_start(
                        out=h_sb[:, c0 * H : c1 * H], in_=h_d[ex, :, c0 * H : c1 * H]
                    )
                wband = wpool.tile([P, WB_F], f32)
                nc.sync.dma_start(out=wband, in_=wband_d[ex])

                # boundary rows: one block-diagonal matmul over the 10
                # consecutive h rows at each block edge
                seam_rhs = srhs_pool.tile([SK, H], f32)
                nc.sync.dma_start(out=seam_rhs, in_=hs_d[ex])
                psum_seam = pseam_pool.tile([SM, H], f32)
                for c0, c1 in ((0, 512), (512, H)):
                    nc.tensor.matmul(
                        out=psum_seam[:, c0:c1],
                        lhsT=wband[0:SK, SEAM_OFF : SEAM_OFF + SM],
                        rhs=seam_rhs[:, c0:c1],
                        start=True,
                        stop=True,
                    )
                seam_sb = ssb_pool.tile([SM, H], f32)
                nc.scalar.copy(out=seam_sb, in_=psum_seam[:])

                out_sb = opool.tile([P, NT * 2 * H], f32)
                # place the boundary rows into the staged output (SWDGE;
                # engine partition-slices must be 32-aligned, DMA is free)
                for t in range(NT):
                    nc.gpsimd.dma_start(
                        out=out_sb[MT:P, t * 2 * H : t * 2 * H + H],
                        in_=seam_sb[SW * t : SW * (t + 1), :],
                    )

                for t in range(NT):
                    rhs = h_sb[:, t * H : (t + 1) * H]
                    lhsT = wband[:, t * MT : (t + 1) * MT]
                    psum = pspool.tile([P, H], f32)
                    # fp32 matmul: moving operand <= 512 cols (one bank)
                    for c0, c1 in ((0, 512), (512, H)):
                        nc.tensor.matmul(
                            out=psum[0:MT, c0:c1],
                            lhsT=lhsT,
                            rhs=rhs[:, c0:c1],
                            start=True,
                            stop=True,
                        )
                    ob = t * 2 * H
                    nc.scalar.copy(out=out_sb[0:MT, ob : ob + H], in_=psum[0:MT, :])
                    wsum = spool.tile([P, 1], f32)
                    nc.vector.reduce_sum(
                        out=wsum,
                        in_=wband[:, BAND_OFF + t * D : BAND_OFF + (t + 1) * D],
                        axis=mybir.AxisListType.X,
                    )
                    nc.vector.tensor_scalar_mul(
                        out=out_sb[:, ob + H : ob + 2 * H],
                        in0=rhs,
                        scalar1=wsum,
                    )
                    if t % (NT // 2) == NT // 2 - 1:
                        # store half the example: 4 blocks, 24KB descriptors
                        c0 = (t - (NT // 2 - 1)) * 2 * H
                        c1 = (t + 1) * 2 * H
                        nc.scalar.dma_start(
                            out=out_d[ex, :, c0:c1], in_=out_sb[:, c0:c1]
                        )
    nc.compile()
    return nc


def _host_prep(span_adjacency, bound_hidden):
    """Extract the used 6-wide diagonal band, build the banded matmul
    weights, and pack h partition-major. Pure gather/layout — no
    arithmetic on the data."""
    adj = span_adjacency.reshape(B, L, L)
    band = np.zeros((B, L, D), dtype=np.float32)
    for d in range(D):
        # band[b, j, d] = adj[b, j, j+d] for j+d < L, else 0
        band[:, : L - d, d] = np.diagonal(adj, offset=d, axis1=1, axis2=2)
    band_t = band.reshape(B, NT, P, D)

    # main lhsT[b, t, k, m] = band[b, 128t+m, k-m] for m < 123 (full windows)
    wa = np.zeros((B, NT, P, MT), dtype=np.float32)
    mm = np.arange(MT)
    for d in range(D):
        wa[:, :, mm + d, mm] = band_t[:, :, :MT, d]

    # seam lhsT[b, 10s+u, 5s+q] = band[b, 128s+123+q, u-q] for 0 <= u-q <= 5
    # (k row 10s+u is h row 128s+123+u; out row m=5s+q is j=128s+123+q)
    seam = np.zeros((B, SK, SM), dtype=np.float32)
    s = np.arange(NT)
    for q in range(SW):
        for u in range(q, q + D):
            seam[:, SR * s + u, SW * s + q] = band_t[:, s, MT + q, u - q]

    wband = np.zeros((B, P, WB_F), dtype=np.float32)
    wband[:, :, :WA_COLS] = wa.transpose(0, 2, 1, 3).reshape(B, P, NT * MT)
    wband[:, :, BAND_OFF:SEAM_OFF] = band_t.transpose(0, 2, 1, 3).reshape(B, P, NT * D)
    wband[:, :SK, SEAM_OFF:] = seam

    h32 = np.ascontiguousarray(bound_hidden, dtype=np.float32)
    # packed h: partition p, block t holds row 128t+p
    hpack = h32.reshape(B, NT, P, H).transpose(0, 2, 1, 3).reshape(B, P, NT * H)
    # seam h rows: 10 consecutive rows 128s+123 .. 128s+132 per boundary
    # (rows >= L are only multiplied by zero weights; use zeros)
    h_pad = np.zeros((B, NT * P + SR, H), dtype=np.float32)
    h_pad[:, :L] = h32
    idx = (P * np.arange(NT)[:, None] + MT + np.arange(SR)[None, :]).ravel()
    hseam = h_pad[:, idx, :]

    return [
        {
            "hpack": np.ascontiguousarray(hpack[BP * c : BP * (c + 1)]),
            "hseam": np.ascontiguousarray(hseam[BP * c : BP * (c + 1)]),
            "wband": np.ascontiguousarray(wband[BP * c : BP * (c + 1)]),
        }
        for c in range(NCORES)
    ]


def _host_unpack(outpacks):
    """outpack [BP, 128, NT*1536] per core -> out [B, L, 1536]."""
    op = np.concatenate(outpacks, axis=0)
    return np.ascontiguousarray(
        op.reshape(B, P, NT, 2 * H).transpose(0, 2, 1, 3).reshape(B, L, 2 * H)
    )


def run(span_adjacency, bound_hidden, trace=False):
    """Run on 8 NeuronCores; returns (out [B, L, 2H] f32, exec_time_ns|None)."""
    global _nc_cache
    from concourse import bass_utils

    in_maps = _host_prep(np.asarray(span_adjacency), np.asarray(bound_hidden))
    if _nc_cache is None:
        _nc_cache = _build_bass()
    res = bass_utils.run_bass_kernel_spmd(
        _nc_cache, in_maps, core_ids=list(range(NCORES)), trace=trace
    )
    out = _host_unpack([r["outpack"] for r in res.results])
    return out, res.exec_time_ns


def kernel(span_adjacency, bound_hidden):
    out, _ = run(span_adjacency, bound_hidden, trace=False)
    return out
